# revision 1
# baseline (speedup 1.0000x reference)
"""AdaptiveMixing kernel distributed over 8 trn2 NeuronCores.

Data-parallel over the B*Q=3600 independent mixing instances (sharding
hint): each core processes 450 instances with the two Linear weights
replicated; outputs are concatenated. No collectives required.
"""

import numpy as np
import jax
import jax.numpy as jnp

# hardcoded problem shapes (self-contained; do not read spec.json)
B, Q = 4, 900
G = 4            # n_groups
P_IN = 32        # in_points
P_OUT = 128      # out_points
C = 64           # eff_in
O = 64           # eff_out
D = 256          # query dim
M_PARAMS = C * O             # 4096
TOTAL = M_PARAMS + P_OUT * P_IN  # 8192
EPS = 1e-5
N_CORES = 8
N = B * Q                    # 3600
NS = N // N_CORES            # 450 per core


def _ln2d(x):
    mu = jnp.mean(x, axis=(-2, -1), keepdims=True)
    var = jnp.mean(jnp.square(x - mu), axis=(-2, -1), keepdims=True)
    return (x - mu) * jax.lax.rsqrt(var + EPS)


def _shard_fn(x, query, Wp, bp, Wo, bo):
    # x: [NS, G, P_IN, C], query: [NS, D]
    n = x.shape[0]
    params = (query @ Wp + bp).reshape(n, G, TOTAL)
    M = params[..., :M_PARAMS].reshape(n, G, C, O)
    S = params[..., M_PARAMS:].reshape(n, G, P_OUT, P_IN)
    out = jnp.einsum('ngpc,ngco->ngpo', x, M)
    out = jax.nn.relu(_ln2d(out))
    out = jnp.einsum('ngqp,ngpo->ngqo', S, out)
    out = jax.nn.relu(_ln2d(out))
    out = out.reshape(n, G * P_OUT * O) @ Wo + bo
    return query + out


_jit_shard = jax.jit(_shard_fn)
_DEVS = jax.devices()[:N_CORES]


def kernel(x, query, Wp, bp, Wo, bo):
    x = np.asarray(x, dtype=np.float32)
    query = np.asarray(query, dtype=np.float32)
    xs = x.reshape(N, G, P_IN, C).reshape(N_CORES, NS, G, P_IN, C)
    qs = query.reshape(N, D).reshape(N_CORES, NS, D)

    # stage weights on every core once; dispatch per-core shards async
    outs = []
    for i, dev in enumerate(_DEVS):
        args = [
            jax.device_put(xs[i], dev),
            jax.device_put(qs[i], dev),
            jax.device_put(Wp, dev),
            jax.device_put(bp, dev),
            jax.device_put(Wo, dev),
            jax.device_put(bo, dev),
        ]
        outs.append(_jit_shard(*args))
    out = np.concatenate([np.asarray(o) for o in outs], axis=0)
    return out.reshape(B, Q, D)
